# revision 33
# baseline (speedup 1.0000x reference)
"""Trainium2 Bass kernel for nn_Net_63342177681543.

Net: h = x @ W.T + b  (Linear 54->54) followed by a DMP trajectory
rollout (301-step scan) -> out (B, 2, 301).

The DMP scan is a linear time-invariant 2x2 recurrence; solving it in
closed form (host, float64) and folding the Linear layer gives

  y[b,d,t] = amp_d[b] * (x_aug[b] . U'_d[:,t]) + y0_d[b] * ag[t]

amp_d = goal-y0 and y0_d are single linear functionals of x, computed
EXACTLY on host (x @ 4 columns of W).  The amp factor is folded into
the stationary matmul operand on host, and y0*ag becomes one extra
contraction row (lhsT row 55 = y0, rhs row 55 = ag).  The device work
per 128-row batch tile is then just:

  PE:   psum[0:301]   = xts0_tile.T @ Z3[:, 0:301]     (K=56, final d0)
        psum[512:813] = xts1_tile.T @ Z3[:, 301:602]   (K=56, final d1)
  ACT:  y[:, 0:301]   = copy(psum[0:301])      (PSUM has no DMA route)
  DVE:  y[:, 301:602] = copy(psum[512:813])
  sync: DMA y -> out   (2408B/partition descriptors)

Sharding: pure data parallel, batch split across 8 cores.
"""

import numpy as np

import concourse.bass as bass
import concourse.mybir as mybir
from concourse.bass_utils import run_bass_kernel_spmd

# ---- problem constants (hardcoded; kernel.py must be self-contained) ----
N = 25
DOF = 2
TAU = 3.0
DT = 0.01
A_Z = 25.0
A_X = 1.0
T = 301           # time steps
B = 65536         # full batch
DIN = 54
N_CORES = 8
B_SHARD = B // N_CORES          # 8192
P = 128                         # partitions / batch tile
N_TILES = B_SHARD // P          # 64
KK = DIN + 2                    # 56: x (54) + amp row + y0 row
OUTC = DOF * T                  # 602 output cols per batch row
CHUNK0 = 4 * P                  # head chunk (cols) so PE starts early
PS_STRIDE = 512                 # d1 block offset in psum (bank aligned)

NB_PSUM = 4                     # psum tiles (2 banks each) = 8 banks
NB_Y = 6                        # output staging buffers

_MM_DT = mybir.dt.bfloat16


def _coeffs():
    """Host precompute of DMP closed-form coefficients (float64)."""
    k = DT / TAU
    q = A_Z * A_Z / 4.0
    A = np.array([[1.0, k], [-k * q, 1.0 - k * A_Z]])
    a = np.empty(T)
    bb = np.empty(T)
    Pm = np.eye(2)
    for t in range(T):
        a[t] = Pm[0, 0]
        bb[t] = Pm[0, 1]
        Pm = A @ Pm
    c = np.exp(-A_X * np.linspace(0.0, 1.0, N))
    sigma2 = (N ** 1.5) / c / A_X
    xph = 1.0
    phi = np.empty((T - 1, N))
    for t in range(T - 1):
        psi = np.exp(-0.5 * (xph - c) ** 2 / sigma2)
        phi[t] = psi * xph / psi.sum()
        xph *= 1.0 - A_X * DT / TAU
    M = np.zeros((N, T))
    g = np.zeros(T)
    for t in range(1, T):
        coef = bb[t - 1 - np.arange(t)]
        M[:, t] = k * (coef @ phi[:t])
        g[t] = k * q * coef.sum()
    return a, g, M


def _host_prep(x, W, b):
    """Z3 (56, 602) rhs and per-core scaled lhsT tensors (56, B_SHARD)."""
    a, g, M = _coeffs()
    W64 = W.astype(np.float64)
    b64 = b.astype(np.float64)
    ag = a + g
    Z3 = np.zeros((KK, DOF * T))
    amp = np.empty((B, DOF), np.float64)
    y0 = np.empty((B, DOF), np.float64)
    x64 = x.astype(np.float64)
    for d in range(DOF):
        Ww = W64[4 + N * d: 4 + N * (d + 1), :]
        bw = b64[4 + N * d: 4 + N * (d + 1)]
        Z3[:DIN, d * T:(d + 1) * T] = Ww.T @ M
        Z3[DIN, d * T:(d + 1) * T] = bw @ M + g       # bias row (+g fold)
        Z3[DIN + 1, d * T:(d + 1) * T] = ag           # y0 row
        amp[:, d] = x64 @ (W64[2 + d] - W64[d]) + (b64[2 + d] - b64[d])
        y0[:, d] = x64 @ W64[d] + b64[d]
    np_dt = mybir.dt.np(_MM_DT)
    Z3c = np.ascontiguousarray(Z3, dtype=np.float32).astype(np_dt)

    xts = []  # per core: [xts_d0, xts_d1]
    for c in range(N_CORES):
        rows = slice(c * B_SHARD, (c + 1) * B_SHARD)
        xs = x64[rows]
        pair = []
        for d in range(DOF):
            m = np.empty((KK, B_SHARD), np.float32)
            m[:DIN] = (xs * amp[rows, d][:, None]).T
            m[DIN] = amp[rows, d]                      # scaled ones row
            m[DIN + 1] = y0[rows, d]
            pair.append(np.ascontiguousarray(m).astype(np_dt))
        xts.append(pair)
    return Z3c, xts


def _build_bass():
    """Raw-Bass SPMD kernel: per core, 64 batch tiles of 128 rows."""
    nc = bass.Bass()
    xt0 = nc.dram_tensor("xt0", [KK, B_SHARD], _MM_DT, kind="ExternalInput")
    xt1 = nc.dram_tensor("xt1", [KK, B_SHARD], _MM_DT, kind="ExternalInput")
    z = nc.dram_tensor("z", [KK, OUTC], _MM_DT, kind="ExternalInput")
    out = nc.dram_tensor("out", [B_SHARD, OUTC], mybir.dt.float32,
                         kind="ExternalOutput")

    from contextlib import ExitStack
    ctx = ExitStack()
    with ctx:
        z_s = ctx.enter_context(nc.sbuf_tensor([KK, OUTC], _MM_DT))
        xa_s = ctx.enter_context(nc.sbuf_tensor([KK, B_SHARD], _MM_DT))
        xb_s = ctx.enter_context(nc.sbuf_tensor([KK, B_SHARD], _MM_DT))
        yb = [ctx.enter_context(
            nc.sbuf_tensor(f"yb{j}", [P, OUTC], mybir.dt.float32))
            for j in range(NB_Y)]
        tp = [ctx.enter_context(
            nc.psum_tensor(f"tp{j}", [P, 2 * PS_STRIDE], mybir.dt.float32))
            for j in range(NB_PSUM)]
        sem_z = ctx.enter_context(nc.semaphore())
        sem_x0 = ctx.enter_context(nc.semaphore())
        sem_xw = ctx.enter_context(nc.semaphore())
        # per-slot DMA sems: completion order across queues is unordered
        sem_out = [ctx.enter_context(nc.semaphore(f"sem_out{j}"))
                   for j in range(NB_Y)]
        sem_pe = ctx.enter_context(nc.semaphore())
        sem_act = ctx.enter_context(nc.semaphore())
        sem_dve = ctx.enter_context(nc.semaphore())
        block = ctx.enter_context(nc.Block())

        @block.gpsimd
        def _(gpsimd):
            # whale loads: everything past the head chunk, one DMA each
            gpsimd.dma_start(out=xa_s[:, CHUNK0:],
                             in_=xt0[:, CHUNK0:]).then_inc(sem_xw, 16)
            gpsimd.dma_start(out=xb_s[:, CHUNK0:],
                             in_=xt1[:, CHUNK0:]).then_inc(sem_xw, 16)

        @block.tensor
        def _(tensor):
            for i in range(N_TILES):
                if i == 0:
                    tensor.wait_ge(sem_z, 16)
                    tensor.wait_ge(sem_x0, 32)
                if i == CHUNK0 // P:
                    tensor.wait_ge(sem_xw, 32)
                if i >= NB_PSUM:
                    # both psum readers done with set i-NB_PSUM
                    tensor.wait_ge(sem_act, i - NB_PSUM + 1)
                    tensor.wait_ge(sem_dve, i - NB_PSUM + 1)
                ps = tp[i % NB_PSUM]
                csl = slice(i * P, (i + 1) * P)
                nc.tensor.matmul(ps[:, 0:T], xa_s[:, csl],
                                 z_s[:, 0:T], start=True, stop=True)
                nc.tensor.matmul(ps[:, PS_STRIDE:PS_STRIDE + T],
                                 xb_s[:, csl], z_s[:, T:OUTC],
                                 start=True, stop=True).then_inc(sem_pe, 1)

        @block.scalar
        def _(scalar):
            copy = mybir.ActivationFunctionType.Copy
            for i in range(N_TILES):
                scalar.wait_ge(sem_pe, i + 1)
                if i >= NB_Y:
                    # y slot free: its previous DMA (tile i-NB_Y) done
                    scalar.wait_ge(sem_out[i % NB_Y], (i // NB_Y) * 16)
                nc.scalar.activation(yb[i % NB_Y][:, 0:T],
                                     tp[i % NB_PSUM][:, 0:T],
                                     copy).then_inc(sem_act, 1)

        @block.vector
        def _(vector):
            for i in range(N_TILES):
                vector.wait_ge(sem_pe, i + 1)
                if i >= NB_Y:
                    vector.wait_ge(sem_out[i % NB_Y], (i // NB_Y) * 16)
                nc.vector.tensor_copy(
                    yb[i % NB_Y][:, T:OUTC],
                    tp[i % NB_PSUM][:, PS_STRIDE:PS_STRIDE + T],
                ).then_inc(sem_dve, 1)

        @block.sync
        def _(sync):
            sync.dma_start(out=z_s[:, :], in_=z[:, :]).then_inc(sem_z, 16)
            sync.dma_start(out=xa_s[:, 0:CHUNK0],
                           in_=xt0[:, 0:CHUNK0]).then_inc(sem_x0, 16)
            sync.dma_start(out=xb_s[:, 0:CHUNK0],
                           in_=xt1[:, 0:CHUNK0]).then_inc(sem_x0, 16)
            for i in range(N_TILES):
                sync.wait_ge(sem_act, i + 1)
                sync.wait_ge(sem_dve, i + 1)
                sync.dma_start(
                    out=out[i * P:(i + 1) * P, :],
                    in_=yb[i % NB_Y][:, :]).then_inc(sem_out[i % NB_Y], 16)
            # kernel must not retire until every output DMA has landed
            for j in range(NB_Y):
                n_dmas = len(range(j, N_TILES, NB_Y))
                sync.wait_ge(sem_out[j], n_dmas * 16)

    return nc


_NC_CACHE = None


def kernel(x, W, b):
    global _NC_CACHE
    x = np.ascontiguousarray(x, dtype=np.float32)
    Z3, xts = _host_prep(x, np.asarray(W), np.asarray(b))
    if _NC_CACHE is None:
        _NC_CACHE = _build_bass()
    nc = _NC_CACHE

    in_maps = [{"xt0": xts[c][0], "xt1": xts[c][1], "z": Z3}
               for c in range(N_CORES)]
    res = run_bass_kernel_spmd(nc, in_maps, list(range(N_CORES)))
    out = np.concatenate([res.results[c]["out"] for c in range(N_CORES)],
                         axis=0)                            # (65536, 602)
    return out.reshape(B, DOF, T)
